# revision 7
# baseline (speedup 1.0000x reference)
"""GPT-2 attention (B=2, S=2048, D=1024, H=16) on 8 TRN2 NeuronCores.

Sharding: 2-way data parallel over batch x 4-way tensor parallel over heads.
Core c handles batch b = c // 4 and heads 4g..4g+3 where g = c % 4.

Per-core kernel (all matmul inputs bf16, fp32 PSUM accumulation):
  1. QKV^T projection: Q^T, K^T computed in [head_dim, seq] layout
     (lhsT = W tiles, rhs = x^T tiles); V computed in natural [seq, head_dim]
     layout (lhsT = x^T tiles, rhs = Wv) with a ones column appended.
  2. Per head: scores^T[sk, sq] = K^T_tile.T @ Q^T (single K=64 matmul per
     tile pair), exp via ScalarE (scale = 1/sqrt(64)) directly PSUM->SBUF,
     causal mask via multiply with one of 4 precomputed [128, 512] masks on
     diagonal blocks only; fully-masked blocks are skipped entirely.
  3. AV: O_aug^T[65, sq] = V_aug.T @ P^T accumulated over sk tiles; row 64
     (from the ones column of V_aug) is the softmax denominator.
  4. Normalization: reciprocal of the denominators, broadcast across 64
     partitions with a K=1 matmul, multiplied into O^T (division commutes
     out of the output projection's contraction).
  5. Output projection: y_partial[sq, 1024] = O_scaled^T.T @ Wproj_shard,
     DMA'd straight from PSUM to DRAM.

Host: x[b].T and weight shards pre-cast to bf16; the 4 per-batch partials
are summed on host (row-split matmul unshard) and bproj added.
"""

import numpy as np
import ml_dtypes

import concourse.bass as bass
import concourse.mybir as mybir
import concourse.tile as tile
from concourse import bacc
from concourse.bass_utils import run_bass_kernel_spmd

BF16 = ml_dtypes.bfloat16

B, S, D, H = 2, 2048, 1024, 16
HD = D // H            # 64
NH = 4                 # heads per core
JQ = NH * HD           # 256 q (or k, or v) columns per core
P = 128
SC = 512               # seq chunk (matmul free dim / PSUM bank)
NSC = S // SC          # 4
NST = S // P           # 16 seq tiles
NDC = D // P           # 8 contraction chunks over model dim
SCALE = 1.0 / np.sqrt(np.float32(HD))  # 0.125

_COMPILED = {}


def build(has_qkv_bias: bool):
    f32 = mybir.dt.float32
    bf16 = mybir.dt.bfloat16
    nc = bacc.Bacc()

    xT = nc.declare_dram_parameter("xT", [D, S], bf16, isOutput=False)
    w = nc.declare_dram_parameter("w", [D, 3 * JQ], bf16, isOutput=False)
    wp = nc.declare_dram_parameter("wp", [JQ, D], bf16, isOutput=False)
    if has_qkv_bias:
        bqkv = nc.declare_dram_parameter("bqkv", [1, 3 * JQ], bf16, isOutput=False)
    y = nc.declare_dram_parameter("y", [S, D], f32, isOutput=True)

    with tile.TileContext(nc) as tc:
        with (
            tc.tile_pool(name="const", bufs=1) as const,
            tc.tile_pool(name="inp", bufs=1) as inp,
            tc.tile_pool(name="qkv", bufs=1) as qkv,
            tc.tile_pool(name="ptp", bufs=2) as ptp,
            tc.tile_pool(name="ps", bufs=8, space="PSUM") as psp,
        ):
            # ---- constants ----
            masks = const.tile([P, NSC, SC], bf16)  # masks[:, m] keeps j >= p + m*128
            nc.gpsimd.memset(masks[:], 1.0)
            for m in range(NSC):
                nc.gpsimd.affine_select(
                    out=masks[:, m, :],
                    in_=masks[:, m, :],
                    compare_op=mybir.AluOpType.is_ge,
                    fill=0.0,
                    base=-m * P,
                    pattern=[[1, SC]],
                    channel_multiplier=-1,
                )
            # ones rows: row 0 feeds bias matmuls, row 64 feeds the
            # denominator-broadcast matmul (lhsT/rhs must share base partition)
            ones = const.tile([P, SC], bf16)
            nc.gpsimd.memset(ones[:], 1.0)

            # ---- load inputs ----
            xT_sb = inp.tile([P, NDC, S], bf16)
            nc.sync.dma_start(out=xT_sb[:], in_=xT[:].rearrange("(a p) s -> p a s", p=P))
            w_sb = inp.tile([P, NDC, 3 * JQ], bf16)
            nc.sync.dma_start(out=w_sb[:], in_=w[:].rearrange("(a p) j -> p a j", p=P))
            wp_sb = inp.tile([P, JQ // P, D], bf16)
            nc.sync.dma_start(out=wp_sb[:], in_=wp[:].rearrange("(a p) j -> p a j", p=P))
            if has_qkv_bias:
                b_sb = inp.tile([1, 3 * JQ], bf16)
                nc.sync.dma_start(out=b_sb[:], in_=bqkv[:])

            # ---- QKV^T: Q^T, K^T in [j, s] layout ----
            qT = qkv.tile([P, 2, S], bf16)  # partitions: head pair (h%2)*64 + hd
            kT = qkv.tile([P, 2, S], bf16)
            for jt in range(4):  # 0,1: q tiles; 2,3: k tiles
                dest, jl = (qT, jt) if jt < 2 else (kT, jt - 2)
                for c in range(NSC):
                    ps_qkv = psp.tile([P, SC], f32, tag="ps", name="ps_qkv")
                    for a in range(NDC):
                        nc.tensor.matmul(
                            ps_qkv[:],
                            lhsT=w_sb[:, a, jt * P:(jt + 1) * P],
                            rhs=xT_sb[:, a, c * SC:(c + 1) * SC],
                            start=(a == 0),
                            stop=(a == NDC - 1) and not has_qkv_bias,
                        )
                    if has_qkv_bias:
                        nc.tensor.matmul(
                            ps_qkv[:],
                            lhsT=b_sb[0:1, jt * P:(jt + 1) * P],
                            rhs=ones[0:1, :SC],
                            start=False,
                            stop=True,
                        )
                    nc.vector.tensor_copy(dest[:, jl, c * SC:(c + 1) * SC], ps_qkv[:])

            # ---- V in [s, hd] layout with ones column (65 cols per head) ----
            v_sb = qkv.tile([P, NST * NH, HD + 1], bf16)
            nc.vector.memset(v_sb[:, :, HD:HD + 1], 1.0)
            for t in range(NST):
                ps_v = psp.tile([P, SC], f32, tag="ps", name="ps_v")
                for a in range(NDC):
                    nc.tensor.matmul(
                        ps_v[:, 0:JQ],
                        lhsT=xT_sb[:, a, t * P:(t + 1) * P],
                        rhs=w_sb[:, a, 2 * JQ:3 * JQ],
                        start=(a == 0),
                        stop=(a == NDC - 1) and not has_qkv_bias,
                    )
                if has_qkv_bias:
                    nc.tensor.matmul(
                        ps_v[:, 0:JQ],
                        lhsT=ones[0:1, 0:P],
                        rhs=b_sb[0:1, 2 * JQ:3 * JQ],
                        start=False,
                        stop=True,
                    )  # ones/b_sb both at base partition 0
                for h in range(NH):
                    nc.scalar.copy(
                        v_sb[:, t * NH + h, 0:HD], ps_v[:, h * HD:(h + 1) * HD]
                    )

            # ---- attention per head ----
            # O^T stored as [128, 2, S]: partition = (h%2)*64 + hd, free = (h//2, s)
            oT = qkv.tile([P, 2, S], bf16)
            # softmax denominator reciprocals, staged on partition 64 (the row
            # the V ones-column writes to); only that partition is used
            recipst = qkv.tile([P, NH * NSC, SC], bf16)

            for h in range(NH):
                jl, po = h // 2, (h % 2) * HD
                for c in range(NSC):
                    nv = min(4 * (c + 1), NST)  # valid sk tiles: t*128 <= c*512+511
                    pt = ptp.tile([P, NST, SC], bf16, tag="pt", name="pt")
                    for t in range(nv):
                        ps_sc = psp.tile([P, SC], f32, tag="ps", name="ps_sc")
                        nc.tensor.matmul(
                            ps_sc[:],
                            lhsT=kT[po:po + HD, jl, t * P:(t + 1) * P],
                            rhs=qT[po:po + HD, jl, c * SC:(c + 1) * SC],
                            start=True,
                            stop=True,
                        )
                        nc.scalar.activation(
                            pt[:, t, :], ps_sc[:],
                            mybir.ActivationFunctionType.Exp, scale=float(SCALE),
                        )
                        if t >= 4 * c:  # diagonal block: mask within tile
                            nc.vector.tensor_mul(
                                pt[:, t, :], pt[:, t, :], masks[:, t - 4 * c, :]
                            )
                    ps_av = psp.tile([P, SC], f32, tag="ps", name="ps_av")
                    for t in range(nv):
                        nc.tensor.matmul(
                            ps_av[0:HD + 1, :],
                            lhsT=v_sb[:, t * NH + h, :],
                            rhs=pt[:, t, :],
                            start=(t == 0),
                            stop=(t == nv - 1),
                        )
                    nc.vector.tensor_copy(
                        oT[po:po + HD, jl, c * SC:(c + 1) * SC], ps_av[0:HD, :]
                    )
                    with nc.allow_low_precision(reason="bf16 softmax denom recip"):
                        nc.vector.reciprocal(
                            recipst[64:65, h * NSC + c, :], ps_av[HD:HD + 1, :]
                        )

            # ---- normalize: O^T *= bcast(1 / sums) ----
            for h in range(NH):
                jl, po = h // 2, (h % 2) * HD
                for c in range(NSC):
                    ps_bc = psp.tile([P, SC], f32, tag="ps", name="ps_bc")
                    nc.tensor.matmul(
                        ps_bc[0:HD, :],
                        lhsT=ones[64:65, 0:HD],
                        rhs=recipst[64:65, h * NSC + c, :],
                        start=True,
                        stop=True,
                    )
                    nc.vector.tensor_mul(
                        oT[po:po + HD, jl, c * SC:(c + 1) * SC],
                        oT[po:po + HD, jl, c * SC:(c + 1) * SC],
                        ps_bc[0:HD, :],
                    )

            # ---- output projection: y = O_scaled^T.T @ wp ----
            for st in range(NST):
                for jc in range(2):
                    ps_y = psp.tile([P, SC], f32, tag="ps", name="ps_y")
                    for cc in range(2):
                        nc.tensor.matmul(
                            ps_y[:],
                            lhsT=oT[:, cc, st * P:(st + 1) * P],
                            rhs=wp_sb[:, cc, jc * SC:(jc + 1) * SC],
                            start=(cc == 0),
                            stop=(cc == 1),
                        )
                    y_sb = ptp.tile([P, SC], f32, tag="ysb", name="y_sb", bufs=4)
                    if (st + jc) % 2 == 0:
                        nc.vector.tensor_copy(y_sb[:], ps_y[:])
                    else:
                        nc.scalar.copy(y_sb[:], ps_y[:])
                    nc.sync.dma_start(
                        out=y[st * P:(st + 1) * P, jc * SC:(jc + 1) * SC], in_=y_sb[:]
                    )

    nc.compile()
    return nc


def get_compiled(has_qkv_bias: bool):
    key = bool(has_qkv_bias)
    if key not in _COMPILED:
        _COMPILED[key] = build(key)
    return _COMPILED[key]


def make_in_maps(x, Wqkv, bqkv, Wproj):
    has_bias = bool(np.any(bqkv))
    xTs = [np.ascontiguousarray(x[b].T).astype(BF16) for b in range(B)]
    in_maps = []
    for c in range(8):
        b, g = c // 4, c % 4
        sl = slice(g * JQ, (g + 1) * JQ)
        wshard = np.concatenate(
            [Wqkv[:, sl], Wqkv[:, D + g * JQ:D + (g + 1) * JQ],
             Wqkv[:, 2 * D + g * JQ:2 * D + (g + 1) * JQ]], axis=1
        ).astype(BF16)
        m = {
            "xT": xTs[b],
            "w": np.ascontiguousarray(wshard),
            "wp": np.ascontiguousarray(Wproj[sl]).astype(BF16),
        }
        if has_bias:
            bshard = np.concatenate(
                [bqkv[sl], bqkv[D + g * JQ:D + (g + 1) * JQ],
                 bqkv[2 * D + g * JQ:2 * D + (g + 1) * JQ]]
            ).astype(BF16)
            m["bqkv"] = np.ascontiguousarray(bshard[None, :])
        in_maps.append(m)
    return has_bias, in_maps


def run(x, Wqkv, bqkv, Wproj, bproj, trace=False):
    has_bias, in_maps = make_in_maps(x, Wqkv, bqkv, Wproj)
    nc = get_compiled(has_bias)
    res = run_bass_kernel_spmd(nc, in_maps, core_ids=list(range(8)), trace=trace)
    out = np.zeros((B, S, D), np.float32)
    for c in range(8):
        out[c // 4] += res.results[c]["y"]
    out += bproj.astype(np.float32)
    return out, res


def kernel(x, Wqkv, bqkv, Wproj, bproj):
    x = np.asarray(x, np.float32)
    Wqkv = np.asarray(Wqkv, np.float32)
    bqkv = np.asarray(bqkv, np.float32)
    Wproj = np.asarray(Wproj, np.float32)
    bproj = np.asarray(bproj, np.float32)
    out, _ = run(x, Wqkv, bqkv, Wproj, bproj, trace=False)
    return out
